# revision 1
# baseline (speedup 1.0000x reference)
"""LIF (leaky integrate-and-fire) forward kernel for Trainium2, 8 NeuronCores.

Recurrence (per element of [B, N], serial over T):
    v_t = DECAY * (v_{t-1} * (1 - s_{t-1})) + x_t      (REST = 0)
    s_t = (v_t > THRESHOLD)

Lanes (columns of the per-core [128, 2048] slab):
  D-lane  cols [0, C_D):        exact fp32 recurrence on DVE (2 fused STTs)
  P1/P2 sub-lanes (C_D..2048):  TensorE computes v in PSUM:
      psv = (DECAY I)^T w  [fp32r, 1cyc/row]  + I^T x_hi + I^T x_lo  [bf16]
      m   = Sign(psv - THR)   ScalarE -> bf16
      w   = (m is_le 0)*psv   DVE STT -> fp32r state
  Two sub-lanes pipeline the PE -> ScalarE -> DVE -> PE chain across steps.

Output: TensorE packs 8 steps of m into a PSUM byte-plane
(psum += 2^k I^T m_k, emitted one step late to fill PE stalls), ScalarE
drains it as int8 (B-128), 1 byte per 8 steps per element.

Input feeds: cols [0,C_D) fp32 plane; cols [C_D,2048) bf16 hi+lo planes
(same bytes/elem as fp32). All per-core tensors are p-major so chunked
DMA loads are contiguous 16-32 KiB per partition.
"""

import numpy as np
import ml_dtypes

import concourse.bacc as bacc
import concourse.mybir as mybir
from concourse.tile import TileContext
from concourse.bass_utils import run_bass_kernel_spmd

T, B, N = 32, 128, 16384
N_CORES = 8
B_SH = B // N_CORES          # 16 batch rows per core
S = B_SH * N                 # 262144 elements per core per time step
P = 128                      # SBUF partitions
F = S // P                   # 2048 free-dim elements
DECAY = 0.2
THR = 0.3

C_D = 384                    # columns on the exact DVE lane
C_G = 128                    # columns on the GpSimd arith lane
C_DG = C_D + C_G             # fp32-fed columns (the "xd" plane)
C_P = F - C_DG               # columns on the TensorE lane
SUB = (512, 512, 512)        # P-lane sub-lane widths (sum == C_P)
CHUNK = 4                    # time steps per input DMA
N_WARM = 0                   # dummy matmuls per step keeping PE un-throttled
GROUPS = T // 8              # byte-planes (8 steps packed per byte)

TRACE = False                # set True (e.g. from test.py) to capture a profile

_BUILT = {}


def _build_nc():
    nc = bacc.Bacc("TRN2", debug=False, num_devices=N_CORES)
    f32 = mybir.dt.float32
    f32r = mybir.dt.float32r
    bf16 = mybir.dt.bfloat16
    Alu = mybir.AluOpType
    Act = mybir.ActivationFunctionType

    xd = nc.dram_tensor("xd", [P, T * C_DG], f32, kind="ExternalInput").ap()
    xh = nc.dram_tensor("xh", [P, T * C_P], bf16, kind="ExternalInput").ap()
    xl = nc.dram_tensor("xl", [P, T * C_P], bf16, kind="ExternalInput").ap()
    wp_in = nc.dram_tensor("wpack", [P, 8 * P], bf16, kind="ExternalInput").ap()
    dec_in = nc.dram_tensor("decayi", [P, P], f32, kind="ExternalInput").ap()
    ib_in = nc.dram_tensor("identb", [P, P], bf16, kind="ExternalInput").ap()
    y = nc.dram_tensor("y", [P, GROUPS * F], mybir.dt.int8,
                       kind="ExternalOutput").ap()
    xdr = xd.rearrange("p (t f) -> p t f", t=T)
    xhr = xh.rearrange("p (t f) -> p t f", t=T)
    xlr = xl.rearrange("p (t f) -> p t f", t=T)
    yr = y.rearrange("p (g f) -> p g f", g=GROUPS)

    sub_off = [0]
    for wdt in SUB:
        sub_off.append(sub_off[-1] + wdt)

    with TileContext(nc) as tc:
        with (
            tc.tile_pool(name="consts", bufs=1) as c_pool,
            tc.tile_pool(name="state", bufs=1) as state_pool,
            tc.tile_pool(name="xin", bufs=3) as xin_pool,
            tc.tile_pool(name="vtmp", bufs=2) as v_pool,
            tc.tile_pool(name="mtile", bufs=3) as m_pool,
            tc.tile_pool(name="outs", bufs=2) as o_pool,
            tc.tile_pool(name="psv1", bufs=1, space="PSUM") as pv1_pool,
            tc.tile_pool(name="psv2", bufs=1, space="PSUM") as pv2_pool,
            tc.tile_pool(name="psv3", bufs=1, space="PSUM") as pv3_pool,
            tc.tile_pool(name="pack", bufs=1, space="PSUM") as pk_pool,
        ):
            negthr = nc.alloc_sbuf_tensor("const_negthr", [P, 1], f32).ap()
            nc.gpsimd.memset(negthr, -THR)
            neghalf = nc.alloc_sbuf_tensor("const_neghalf", [P, 1], f32).ap()
            nc.gpsimd.memset(neghalf, -0.5)

            wsb = c_pool.tile([P, 8 * P], bf16)
            nc.sync.dma_start(out=wsb[:], in_=wp_in)
            decf = c_pool.tile([P, P], f32)
            nc.sync.dma_start(out=decf[:], in_=dec_in)
            identb = c_pool.tile([P, P], bf16)
            nc.sync.dma_start(out=identb[:], in_=ib_in)
            decr = c_pool.tile([P, P], f32r)
            nc.vector.scalar_tensor_tensor(
                out=decr[:], in0=decf[:], scalar=1.0, in1=decf[:],
                op0=Alu.bypass, op1=Alu.bypass,
            )

            wd = state_pool.tile([P, max(C_D, 1)], f32)   # D-lane state
            wg = state_pool.tile([P, max(C_G, 1)], f32)   # G-lane state
            msg = state_pool.tile([P, max(C_G, 1)], f32)  # G-lane mask
            wp = state_pool.tile([P, C_P], f32r)      # P-lane state (rounded)

            pv_pools = (pv1_pool, pv2_pool, pv3_pool)

            def emit_pack(m_prev, t_prev):
                k = t_prev % 8
                wk = wsb[:, k * P:(k + 1) * P]
                for r in range(0, F, 512):
                    nc.tensor.matmul(
                        out=pack_psum[:, r:r + 512], lhsT=wk,
                        rhs=m_prev[:, r:r + 512],
                        start=(k == 0), stop=(k == 7),
                    )

            def emit_drain(t_prev):
                g8 = t_prev // 8
                oi = o_pool.tile([P, F], mybir.dt.int8, name="oi")
                nc.scalar.activation(
                    oi[:], pack_psum[:], Act.Identity, bias=neghalf, scale=0.5)
                nc.scalar.dma_start(out=yr[:, g8, :], in_=oi[:])

            xdt = xht = xlt = None
            m_prev = None
            pack_psum = None
            for t in range(T):
                j = t % CHUNK
                if j == 0:
                    xdt = xin_pool.tile([P, CHUNK * C_DG], f32, name="xdt")
                    xht = xin_pool.tile([P, CHUNK * C_P], bf16, name="xht")
                    xlt = xin_pool.tile([P, CHUNK * C_P], bf16, name="xlt")
                    if t == 0:
                        for jj in range(CHUNK):
                            nc.sync.dma_start(
                                out=xht[:, jj * C_P:(jj + 1) * C_P],
                                in_=xhr[:, jj, :])
                            nc.sync.dma_start(
                                out=xlt[:, jj * C_P:(jj + 1) * C_P],
                                in_=xlr[:, jj, :])
                            nc.sync.dma_start(
                                out=xdt[:, jj * C_DG:(jj + 1) * C_DG],
                                in_=xdr[:, jj, :])
                    else:
                        nc.sync.dma_start(out=xht[:], in_=xhr[:, t:t + CHUNK, :])
                        nc.sync.dma_start(out=xlt[:], in_=xlr[:, t:t + CHUNK, :])
                        nc.sync.dma_start(out=xdt[:], in_=xdr[:, t:t + CHUNK, :])
                xds = xdt[:, j * C_DG:(j + 1) * C_DG]
                xgs = xds[:, C_D:]
                xhs = xht[:, j * C_P:(j + 1) * C_P]
                xls = xlt[:, j * C_P:(j + 1) * C_P]

                if t % 8 == 0:
                    pack_psum = pk_pool.tile([P, F], f32, name="pk")

                v = v_pool.tile([P, C_DG], f32, name="vt")
                m = m_pool.tile([P, F], bf16, name="mt")
                psvs = [
                    pool.tile([P, wdt], f32, name=f"ps{i}")
                    for i, (pool, wdt) in enumerate(zip(pv_pools, SUB))
                ]

                # deferred pack of the previous step fills PE stall time
                if m_prev is not None:
                    emit_pack(m_prev, t - 1)

                # --- P sub-lanes: x-matmuls first (DMA-dependent only),
                # decr last (waits on the previous reset), then Sign ---
                for i, wdt in enumerate(SUB):
                    o0 = sub_off[i]
                    psv = psvs[i]
                    for g in range(0, wdt, 512):
                        a, bnd = o0 + g, o0 + min(g + 512, wdt)
                        nc.tensor.matmul(
                            out=psv[:, g:bnd - o0], lhsT=identb[:],
                            rhs=xhs[:, a:bnd], start=True, stop=False)
                    for g in range(0, wdt, 512):
                        a, bnd = o0 + g, o0 + min(g + 512, wdt)
                        nc.tensor.matmul(
                            out=psv[:, g:bnd - o0], lhsT=identb[:],
                            rhs=xls[:, a:bnd], start=False, stop=(t == 0))
                    if t > 0:
                        for g in range(0, wdt, 512):
                            a, bnd = o0 + g, o0 + min(g + 512, wdt)
                            nc.tensor.matmul(
                                out=psv[:, g:bnd - o0], lhsT=decr[:],
                                rhs=wp[:, a:bnd], start=False, stop=True)
                    nc.scalar.activation(
                        m[:, C_DG + o0:C_DG + o0 + wdt], psv[:],
                        Act.Sign, bias=negthr)
                    if i == 0:
                        # D-lane DVE work runs while ACT handles P1's Sign
                        if C_D > 0 and t == 0:
                            nc.vector.scalar_tensor_tensor(
                                out=wd[:], in0=xds[:, :C_D], scalar=THR,
                                in1=xds[:, :C_D], op0=Alu.is_le, op1=Alu.mult)
                        elif C_D > 0:
                            nc.vector.scalar_tensor_tensor(
                                out=v[:, :C_D], in0=wd[:], scalar=DECAY,
                                in1=xds[:, :C_D], op0=Alu.mult, op1=Alu.add)
                            nc.vector.scalar_tensor_tensor(
                                out=wd[:], in0=v[:, :C_D], scalar=THR,
                                in1=v[:, :C_D], op0=Alu.is_le, op1=Alu.mult)
                        # G-lane: GpSimd arith + DVE fast mask
                        if C_G > 0:
                            if t == 0:
                                vg = xgs
                            else:
                                nc.gpsimd.tensor_tensor(
                                    out=v[:, C_D:], in0=wg[:], in1=xgs,
                                    op=Alu.add)
                                vg = v[:, C_D:]
                            nc.vector.tensor_scalar(
                                out=msg[:], in0=vg, scalar1=THR,
                                scalar2=DECAY, op0=Alu.is_le, op1=Alu.mult)
                            nc.gpsimd.tensor_tensor(
                                out=wg[:], in0=msg[:], in1=vg, op=Alu.mult)

                # P sub-lane resets on DVE
                for i, wdt in enumerate(SUB):
                    o0 = sub_off[i]
                    nc.vector.scalar_tensor_tensor(
                        out=wp[:, o0:o0 + wdt], in0=m[:, C_DG + o0:C_DG + o0 + wdt],
                        scalar=0.0, in1=psvs[i][:], op0=Alu.is_le, op1=Alu.mult)

                # D+G lane Sign (ACT queue: after P sub-lane Signs)
                nc.scalar.activation(
                    m[:, :C_DG], xds if t == 0 else v[:], Act.Sign, bias=negthr)

                # group drain (once per 8 steps), ordered last on ScalarE
                if t > 0 and (t - 1) % 8 == 7:
                    emit_drain(t - 1)

                m_prev = m

            emit_pack(m_prev, T - 1)
            emit_drain(T - 1)
    nc.compile()
    return nc


LAST_RESULTS = None


def _make_consts():
    wp = np.zeros((P, 8 * P), dtype=np.float32)
    for k in range(8):
        wp[:, k * P:(k + 1) * P][np.arange(P), np.arange(P)] = float(2 ** k)
    wpack = (wp.view(np.uint32) >> 16).astype(np.uint16)
    dec = np.zeros((P, P), dtype=np.float32)
    dec[np.arange(P), np.arange(P)] = np.float32(DECAY)
    ib = np.zeros((P, P), dtype=ml_dtypes.bfloat16)
    ib[np.arange(P), np.arange(P)] = 1.0
    return wpack, dec, ib.view(np.uint16)


def kernel(tx):
    global LAST_RESULTS
    tx = np.asarray(tx)
    assert tx.shape == (T, B, N) and tx.dtype == np.float32

    if "nc" not in _BUILT:
        _BUILT["nc"] = _build_nc()
    nc = _BUILT["nc"]

    wpack, dec, ib = _make_consts()
    in_maps = []
    for c in range(N_CORES):
        xc = tx[:, c * B_SH:(c + 1) * B_SH, :].reshape(T, P, F)
        xc = np.ascontiguousarray(xc.transpose(1, 0, 2))     # [P, T, F]
        xdc = np.ascontiguousarray(xc[:, :, :C_DG]).reshape(P, T * C_DG)
        xpc = xc[:, :, C_DG:]
        xhc = xpc.astype(ml_dtypes.bfloat16)
        xlc = (xpc - xhc.astype(np.float32)).astype(ml_dtypes.bfloat16)
        in_maps.append({
            "xd": xdc,
            "xh": np.ascontiguousarray(xhc).reshape(P, T * C_P).view(np.uint16),
            "xl": np.ascontiguousarray(xlc).reshape(P, T * C_P).view(np.uint16),
            "wpack": wpack, "decayi": dec, "identb": ib,
        })

    res = run_bass_kernel_spmd(nc, in_maps, core_ids=list(range(N_CORES)), trace=TRACE)
    LAST_RESULTS = res

    out = np.empty((T, B, N), dtype=np.float32)
    for c in range(N_CORES):
        yb = np.asarray(res.results[c]["y"]).astype(np.int16)  # [P, GROUPS*F]
        Bv = (yb + 128).astype(np.uint8).reshape(P, GROUPS, F)
        for g in range(GROUPS):
            for k in range(8):
                bits = (Bv[:, g, :] >> k) & 1          # [P, F]
                st = bits.reshape(B_SH, N).astype(np.float32)
                out[g * 8 + k, c * B_SH:(c + 1) * B_SH, :] = st
    return out



# revision 5
# speedup vs baseline: 1.2430x; 1.2430x over previous
"""LIF (leaky integrate-and-fire) forward kernel for Trainium2, 8 NeuronCores.

Recurrence (per element of [B, N], serial over T):
    v_t = DECAY * (v_{t-1} * (1 - s_{t-1})) + x_t      (REST = 0)
    s_t = (v_t > THRESHOLD)

v3 design: x is fed in fp16 (2 B/elem -> DMA floor ~47us/core) and the
per-core [128, 2048] slab is split into three lane types:

  P-lane cols [0, C_P):       TensorE accumulates v in PSUM
      psv = I_f16^T x_f16  +  (DECAY I)_f32r^T w_f32r
      m   = Sign(psv - THR)            ScalarE -> bf16 (+-1)
      w   = (psv <= THR) * psv         DVE STT -> fp32 state
  E-lane cols [C_P, C_P+C_E): all-fp16 DVE ops (2x/4x perf modes)
      v = u + x                        DVE tensor_tensor  (2x)
      m = Sign(v - THR)                ScalarE -> bf16 (+-1)
      q = (v <= THR) * DECAY           DVE tensor_scalar (4x)
      u = q * v                        DVE tensor_tensor  (2x)
  G-lane cols [C_P+C_E, F):   GpSimd does the adds/mults
      vg = u_G + x   (GpSimd)          m = (vg > THR) DVE ts (4x) -> {1,0}
      qg = (vg <= THR)*DECAY (DVE 4x)  u_G = qg * vg  (GpSimd)

Output: TensorE packs 8 steps of m into a PSUM byte-plane
(psum += 2^k I^T m_k, emitted one step late), ScalarE drains it as
int8, 1 byte per 8 steps per element.  m is +-1 on P/E cols (drain
affine 0.5x-0.5) and {1,0} on G cols (drain affine x-128).
"""

import numpy as np
import ml_dtypes

import concourse.bacc as bacc
import concourse.mybir as mybir
from concourse.tile import TileContext
from concourse.bass_utils import run_bass_kernel_spmd

T, B, N = 32, 128, 16384
N_CORES = 8
B_SH = B // N_CORES          # 16 batch rows per core
S = B_SH * N                 # 262144 elements per core per time step
P = 128                      # SBUF partitions
F = S // P                   # 2048 free-dim elements
DECAY = 0.2
THR = 0.3

C_P = 1024                   # PSUM-lane columns (must be multiple of 512)
C_E = 640                    # fp16 DVE-lane columns
C_G = F - C_P - C_E          # GpSimd-lane columns
N_PSUB = C_P // 512          # psv sub-lanes
CHUNK = 4                    # time steps per input DMA
GROUPS = T // 8              # byte-planes (8 steps packed per byte)

TRACE = False                # set True (e.g. from test.py) to capture a profile

_BUILT = {}


def _build_nc():
    nc = bacc.Bacc("TRN2", debug=False, num_devices=N_CORES)
    f16 = mybir.dt.float16
    f32 = mybir.dt.float32
    f32r = mybir.dt.float32r
    bf16 = mybir.dt.bfloat16
    i8 = mybir.dt.int8
    Alu = mybir.AluOpType
    Act = mybir.ActivationFunctionType

    xs = nc.dram_tensor("xs", [P, T * F], f16, kind="ExternalInput").ap()
    wp_in = nc.dram_tensor("wpack", [P, 8 * P], bf16, kind="ExternalInput").ap()
    dec_in = nc.dram_tensor("decayi", [P, P], f32, kind="ExternalInput").ap()
    ih_in = nc.dram_tensor("identh", [P, P], f16, kind="ExternalInput").ap()
    y = nc.dram_tensor("y", [P, GROUPS * F], i8, kind="ExternalOutput").ap()
    xr = xs.rearrange("p (t f) -> p t f", t=T)
    yr = y.rearrange("p (g f) -> p g f", g=GROUPS)

    E0 = C_P                 # E-lane column offset
    G0 = C_P + C_E           # G-lane column offset

    with TileContext(nc) as tc:
        with (
            tc.tile_pool(name="consts", bufs=1) as c_pool,
            tc.tile_pool(name="state", bufs=1) as st_pool,
            tc.tile_pool(name="xin", bufs=3) as xin_pool,
            tc.tile_pool(name="ve", bufs=3) as ve_pool,
            tc.tile_pool(name="qe", bufs=2) as qe_pool,
            tc.tile_pool(name="vg", bufs=2) as vg_pool,
            tc.tile_pool(name="qg", bufs=2) as qg_pool,
            tc.tile_pool(name="mtile", bufs=3) as m_pool,
            tc.tile_pool(name="outs", bufs=2) as o_pool,
            tc.tile_pool(name="psva", bufs=2, space="PSUM") as pva_pool,
            tc.tile_pool(name="psvb", bufs=2, space="PSUM") as pvb_pool,
            tc.tile_pool(name="pack", bufs=1, space="PSUM") as pk_pool,
        ):
            negthr = nc.alloc_sbuf_tensor("const_negthr", [P, 1], f32).ap()
            nc.gpsimd.memset(negthr, -THR)
            neghalf = nc.alloc_sbuf_tensor("const_neghalf", [P, 1], f32).ap()
            nc.gpsimd.memset(neghalf, -0.5)
            negb128 = nc.alloc_sbuf_tensor("const_negb128", [P, 1], f32).ap()
            nc.gpsimd.memset(negb128, -128.0)

            wsb = c_pool.tile([P, 8 * P], bf16)
            nc.sync.dma_start(out=wsb[:], in_=wp_in)
            identh = c_pool.tile([P, P], f16)
            nc.sync.dma_start(out=identh[:], in_=ih_in)
            decf = c_pool.tile([P, P], f32)
            nc.sync.dma_start(out=decf[:], in_=dec_in)
            decr = c_pool.tile([P, P], f32r)
            nc.vector.scalar_tensor_tensor(
                out=decr[:], in0=decf[:], scalar=1.0, in1=decf[:],
                op0=Alu.bypass, op1=Alu.bypass,
            )

            w_p = st_pool.tile([P, max(C_P, 1)], f32r)   # P-lane state
            u_e = st_pool.tile([P, max(C_E, 1)], f16)    # E-lane state
            u_g = st_pool.tile([P, max(C_G, 1)], f16)    # G-lane state

            pv_pools = (pva_pool, pvb_pool)

            def emit_pack(m_prev, t_prev):
                k = t_prev % 8
                wk = wsb[:, k * P:(k + 1) * P]
                for r in range(0, F, 512):
                    nc.tensor.matmul(
                        out=pack_psum[:, r:r + 512], lhsT=wk,
                        rhs=m_prev[:, r:r + 512],
                        start=(k == 0), stop=(k == 7),
                    )

            def emit_drain(t_prev):
                g8 = t_prev // 8
                oi = o_pool.tile([P, F], i8, name="oi")
                nc.scalar.activation(
                    oi[:, :G0], pack_psum[:, :G0], Act.Identity,
                    bias=neghalf, scale=0.5)
                if C_G > 0:
                    nc.scalar.activation(
                        oi[:, G0:], pack_psum[:, G0:], Act.Identity,
                        bias=negb128, scale=1.0)
                nc.scalar.dma_start(out=yr[:, g8, :], in_=oi[:])

            xt = None
            m_prev = None
            pack_psum = None
            for t in range(T):
                j = t % CHUNK
                if j == 0:
                    xt = xin_pool.tile([P, CHUNK * F], f16, name="xt")
                    if t == 0:
                        for jj in range(CHUNK):
                            nc.sync.dma_start(
                                out=xt[:, jj * F:(jj + 1) * F],
                                in_=xr[:, jj, :])
                    else:
                        nc.sync.dma_start(out=xt[:], in_=xr[:, t:t + CHUNK, :])
                xp = xt[:, j * F:j * F + C_P]            # P-lane x
                xe = xt[:, j * F + E0:j * F + E0 + C_E]  # E-lane x
                xg = xt[:, j * F + G0:j * F + G0 + C_G]  # G-lane x

                if t % 8 == 0:
                    pack_psum = pk_pool.tile([P, F], f32, name="pk")

                m = m_pool.tile([P, F], bf16, name="mt")
                psvs = [pool.tile([P, 512], f32, name=f"ps{i}")
                        for i, pool in enumerate(pv_pools)]

                # --- PE: x-feed matmuls (DMA-dependent only) ---
                for i in range(N_PSUB):
                    nc.tensor.matmul(
                        out=psvs[i][:], lhsT=identh[:],
                        rhs=xp[:, i * 512:(i + 1) * 512],
                        start=True, stop=(t == 0))
                # deferred pack of the previous step fills PE stall time
                if m_prev is not None:
                    emit_pack(m_prev, t - 1)
                # decay matmuls (wait on previous reset)
                if t > 0:
                    for i in range(N_PSUB):
                        nc.tensor.matmul(
                            out=psvs[i][:], lhsT=decr[:],
                            rhs=w_p[:, i * 512:(i + 1) * 512],
                            start=False, stop=True)

                # --- ScalarE: P-lane Sign (+-1) ---
                for i in range(N_PSUB):
                    nc.scalar.activation(
                        m[:, i * 512:(i + 1) * 512], psvs[i][:],
                        Act.Sign, bias=negthr)

                # --- DVE: P-lane reset into SBUF fp32 state ---
                # (walrus: only one PSUM operand per DVE op, so the mask
                # comes from the Sign output m (+-1, SBUF) like v1 did)
                for i in range(N_PSUB):
                    nc.vector.scalar_tensor_tensor(
                        out=w_p[:, i * 512:(i + 1) * 512],
                        in0=m[:, i * 512:(i + 1) * 512],
                        scalar=0.0, in1=psvs[i][:],
                        op0=Alu.is_le, op1=Alu.mult)

                # --- E-lane (all fp16, DVE fast modes) ---
                if C_E > 0:
                    if t == 0:
                        ve = xe
                    else:
                        vet = ve_pool.tile([P, C_E], f16, name="ve")
                        nc.vector.tensor_tensor(
                            out=vet[:], in0=u_e[:], in1=xe, op=Alu.add)
                        ve = vet[:]
                    nc.scalar.activation(
                        m[:, E0:E0 + C_E], ve, Act.Sign, bias=negthr)
                    qe = qe_pool.tile([P, C_E], f16, name="qe")
                    nc.vector.tensor_scalar(
                        out=qe[:], in0=ve, scalar1=THR, scalar2=DECAY,
                        op0=Alu.is_le, op1=Alu.mult)
                    nc.vector.tensor_tensor(
                        out=u_e[:], in0=qe[:], in1=ve, op=Alu.mult)

                # --- G-lane (GpSimd adds/mults, DVE masks) ---
                if C_G > 0:
                    if t == 0:
                        vg = xg
                    else:
                        vgt = vg_pool.tile([P, C_G], f16, name="vg")
                        nc.gpsimd.tensor_tensor(
                            out=vgt[:], in0=u_g[:], in1=xg, op=Alu.add)
                        vg = vgt[:]
                    nc.vector.tensor_scalar(
                        out=m[:, G0:], in0=vg, scalar1=THR, scalar2=None,
                        op0=Alu.is_gt)
                    qg = qg_pool.tile([P, C_G], f16, name="qg")
                    nc.vector.tensor_scalar(
                        out=qg[:], in0=vg, scalar1=THR, scalar2=DECAY,
                        op0=Alu.is_le, op1=Alu.mult)
                    nc.gpsimd.tensor_tensor(
                        out=u_g[:], in0=qg[:], in1=vg, op=Alu.mult)

                # group drain (once per 8 steps), ordered last on ScalarE
                if t > 0 and (t - 1) % 8 == 7:
                    emit_drain(t - 1)

                m_prev = m

            emit_pack(m_prev, T - 1)
            emit_drain(T - 1)
    nc.compile()
    return nc


LAST_RESULTS = None


def _make_consts():
    wp = np.zeros((P, 8 * P), dtype=np.float32)
    for k in range(8):
        wp[:, k * P:(k + 1) * P][np.arange(P), np.arange(P)] = float(2 ** k)
    wpack = (wp.view(np.uint32) >> 16).astype(np.uint16)
    dec = np.zeros((P, P), dtype=np.float32)
    dec[np.arange(P), np.arange(P)] = np.float32(DECAY)
    ih = np.zeros((P, P), dtype=np.float16)
    ih[np.arange(P), np.arange(P)] = np.float16(1.0)
    return wpack, dec, ih.view(np.uint16)


def kernel(tx):
    global LAST_RESULTS
    tx = np.asarray(tx)
    assert tx.shape == (T, B, N) and tx.dtype == np.float32

    if "nc" not in _BUILT:
        _BUILT["nc"] = _build_nc()
    nc = _BUILT["nc"]

    wpack, dec, ih = _make_consts()
    in_maps = []
    for c in range(N_CORES):
        xc = tx[:, c * B_SH:(c + 1) * B_SH, :].reshape(T, P, F)
        xc = np.ascontiguousarray(xc.transpose(1, 0, 2))     # [P, T, F]
        xh = xc.astype(np.float16).reshape(P, T * F)
        in_maps.append({
            "xs": xh.view(np.uint16),
            "wpack": wpack, "decayi": dec, "identh": ih,
        })

    res = run_bass_kernel_spmd(nc, in_maps, core_ids=list(range(N_CORES)),
                               trace=TRACE)
    LAST_RESULTS = res

    out = np.empty((T, B, N), dtype=np.float32)
    for c in range(N_CORES):
        yb = np.asarray(res.results[c]["y"]).astype(np.int16)  # [P, GROUPS*F]
        Bv = (yb + 128).astype(np.uint8).reshape(P, GROUPS, F)
        for g in range(GROUPS):
            for k in range(8):
                bits = (Bv[:, g, :] >> k) & 1          # [P, F]
                st = bits.reshape(B_SH, N).astype(np.float32)
                out[g * 8 + k, c * B_SH:(c + 1) * B_SH, :] = st
    return out


# revision 6
# speedup vs baseline: 1.4614x; 1.1757x over previous
"""LIF (leaky integrate-and-fire) forward kernel for Trainium2, 8 NeuronCores.

Recurrence (per element of [B, N], serial over T):
    v_t = DECAY * (v_{t-1} * (1 - s_{t-1})) + x_t      (REST = 0)
    s_t = (v_t > THRESHOLD)

v3.2 design: x is fed in fp16 (2 B/elem -> DMA floor ~47us/core).  The
carried state is u = DECAY * v * (v <= THR), in fp16, so the update is
v' = u + x.  The per-step spike indicator is q = (v <= THR) * DECAY,
which is 0 on spike and ~0.2 otherwise; q doubles as both the reset
multiplier (u = q * v) and the packed output source, so no Sign pass
is needed anywhere.

Two lane types over the per-core [128, 2048] slab:
  S-lane cols [0, C_S):    TensorE does the add in PSUM
      psv = I_f16^T x_f16 + I_f16^T u_f16      (2 matmuls)
      v   = Identity(psv)     ScalarE -> fp16 SBUF
      q   = (v <= THR)*DECAY  DVE tensor_scalar (4x mode)
      u   = q * v             DVE tensor_tensor (2x mode)
  E-lane cols [C_S, F):    pure fp16 DVE
      v = u + x               DVE tensor_tensor (2x)
      q = (v <= THR)*DECAY    DVE tensor_scalar (4x)
      u = q * v               DVE tensor_tensor (2x)

Output: TensorE packs 8 steps of q into a PSUM byte-plane
(psum += 5*2^k I^T q_k ~= 2^k (1 - s_k)), one step deferred; ScalarE
drains it as int8 via out = -psum + 127 = S - 128 where
S = sum 2^k s_k.  Host adds 128 and unpacks bits.
"""

import numpy as np

import concourse.bacc as bacc
import concourse.mybir as mybir
from concourse.tile import TileContext
from concourse.bass_utils import run_bass_kernel_spmd

T, B, N = 32, 128, 16384
N_CORES = 8
B_SH = B // N_CORES          # 16 batch rows per core
S = B_SH * N                 # 262144 elements per core per time step
P = 128                      # SBUF partitions
F = S // P                   # 2048 free-dim elements
DECAY = 0.2
THR = 0.3

C_S = 1024                   # PSUM-lane columns (multiple of 512)
C_E = F - C_S                # fp16 DVE-lane columns
N_SSUB = C_S // 512          # psv sub-lanes
CHUNK = 4                    # time steps per input DMA
GROUPS = T // 8              # byte-planes (8 steps packed per byte)

TRACE = False                # set True (e.g. from test.py) to capture a profile

_BUILT = {}


def _build_nc():
    nc = bacc.Bacc("TRN2", debug=False, num_devices=N_CORES)
    f16 = mybir.dt.float16
    f32 = mybir.dt.float32
    i8 = mybir.dt.int8
    Alu = mybir.AluOpType
    Act = mybir.ActivationFunctionType

    xs = nc.dram_tensor("xs", [P, T * F], f16, kind="ExternalInput").ap()
    wp_in = nc.dram_tensor("wpack", [P, 8 * P], f16, kind="ExternalInput").ap()
    ih_in = nc.dram_tensor("identh", [P, P], f16, kind="ExternalInput").ap()
    y = nc.dram_tensor("y", [P, GROUPS * F], i8, kind="ExternalOutput").ap()
    xr = xs.rearrange("p (t f) -> p t f", t=T)
    yr = y.rearrange("p (g f) -> p g f", g=GROUPS)

    E0 = C_S                 # E-lane column offset

    with TileContext(nc) as tc:
        with (
            tc.tile_pool(name="consts", bufs=1) as c_pool,
            tc.tile_pool(name="state", bufs=1) as st_pool,
            tc.tile_pool(name="xin", bufs=3) as xin_pool,
            tc.tile_pool(name="vs", bufs=3) as vs_pool,
            tc.tile_pool(name="qs", bufs=3) as qs_pool,
            tc.tile_pool(name="ve", bufs=3) as ve_pool,
            tc.tile_pool(name="qe", bufs=3) as qe_pool,
            tc.tile_pool(name="outs", bufs=2) as o_pool,
            tc.tile_pool(name="psva", bufs=2, space="PSUM") as pva_pool,
            tc.tile_pool(name="psvb", bufs=2, space="PSUM") as pvb_pool,
            tc.tile_pool(name="pack", bufs=1, space="PSUM") as pk_pool,
        ):
            zerob = nc.alloc_sbuf_tensor("const_zerob", [P, 1], f32).ap()
            nc.gpsimd.memset(zerob, 0.0)
            pos127 = nc.alloc_sbuf_tensor("const_pos127", [P, 1], f32).ap()
            nc.gpsimd.memset(pos127, 127.0)

            wsb = c_pool.tile([P, 8 * P], f16)
            nc.sync.dma_start(out=wsb[:], in_=wp_in)
            identh = c_pool.tile([P, P], f16)
            nc.sync.dma_start(out=identh[:], in_=ih_in)

            u_s = st_pool.tile([P, max(C_S, 1)], f16)    # S-lane state
            u_e = st_pool.tile([P, max(C_E, 1)], f16)    # E-lane state

            pv_pools = (pva_pool, pvb_pool)

            def emit_pack(q_tiles, t_prev):
                # q_tiles: list of (ap, width) covering the F columns of step
                # t_prev in order
                k = t_prev % 8
                wk = wsb[:, k * P:(k + 1) * P]
                col = 0
                for ap, wdt in q_tiles:
                    for r in range(0, wdt, 512):
                        nc.tensor.matmul(
                            out=pack_psum[:, col + r:col + r + 512], lhsT=wk,
                            rhs=ap[:, r:r + 512],
                            start=(k == 0), stop=(k == 7),
                        )
                    col += wdt

            def emit_drain(t_prev):
                g8 = t_prev // 8
                oi = o_pool.tile([P, F], i8, name="oi")
                nc.scalar.activation(
                    oi[:], pack_psum[:], Act.Identity,
                    bias=pos127, scale=-1.0)
                nc.scalar.dma_start(out=yr[:, g8, :], in_=oi[:])

            xt = None
            q_prev = None
            pack_psum = None
            for t in range(T):
                j = t % CHUNK
                if j == 0:
                    xt = xin_pool.tile([P, CHUNK * F], f16, name="xt")
                    if t == 0:
                        for jj in range(CHUNK):
                            nc.sync.dma_start(
                                out=xt[:, jj * F:(jj + 1) * F],
                                in_=xr[:, jj, :])
                    else:
                        nc.sync.dma_start(out=xt[:], in_=xr[:, t:t + CHUNK, :])
                xsl = [xt[:, j * F + i * 512:j * F + (i + 1) * 512]
                       for i in range(N_SSUB)]
                xe = xt[:, j * F + E0:j * F + E0 + C_E]  # E-lane x

                if t % 8 == 0:
                    pack_psum = pk_pool.tile([P, F], f32, name="pk")

                psvs = [pool.tile([P, 512], f32, name=f"ps{i}")
                        for i, pool in enumerate(pv_pools)]

                # --- PE: x-feed matmuls (DMA-dependent only) ---
                for i in range(N_SSUB):
                    nc.tensor.matmul(
                        out=psvs[i][:], lhsT=identh[:], rhs=xsl[i],
                        start=True, stop=(t == 0))
                # deferred pack of the previous step fills PE stall time
                if q_prev is not None:
                    emit_pack(q_prev, t - 1)
                # u-feed matmuls (wait on previous reset)
                if t > 0:
                    for i in range(N_SSUB):
                        nc.tensor.matmul(
                            out=psvs[i][:], lhsT=identh[:],
                            rhs=u_s[:, i * 512:(i + 1) * 512],
                            start=False, stop=True)

                # --- S-lane: ScalarE copy + DVE fast ops ---
                qss = []
                for i in range(N_SSUB):
                    vs = vs_pool.tile([P, 512], f16, name=f"vs{i}")
                    nc.scalar.activation(
                        vs[:], psvs[i][:], Act.Identity, bias=zerob)
                    qs = qs_pool.tile([P, 512], f16, name=f"qs{i}")
                    nc.vector.tensor_scalar(
                        out=qs[:], in0=vs[:], scalar1=THR, scalar2=DECAY,
                        op0=Alu.is_le, op1=Alu.mult)
                    nc.vector.tensor_tensor(
                        out=u_s[:, i * 512:(i + 1) * 512], in0=qs[:],
                        in1=vs[:], op=Alu.mult)
                    qss.append((qs, 512))

                # --- E-lane (all fp16, DVE fast modes) ---
                if t == 0:
                    ve = xe
                else:
                    vet = ve_pool.tile([P, C_E], f16, name="ve")
                    nc.vector.tensor_tensor(
                        out=vet[:], in0=u_e[:], in1=xe, op=Alu.add)
                    ve = vet[:]
                qe = qe_pool.tile([P, C_E], f16, name="qe")
                nc.vector.tensor_scalar(
                    out=qe[:], in0=ve, scalar1=THR, scalar2=DECAY,
                    op0=Alu.is_le, op1=Alu.mult)
                nc.vector.tensor_tensor(
                    out=u_e[:], in0=qe[:], in1=ve, op=Alu.mult)

                # group drain (once per 8 steps), ordered last on ScalarE
                if t > 0 and (t - 1) % 8 == 7:
                    emit_drain(t - 1)

                q_prev = qss + [(qe, C_E)]

            emit_pack(q_prev, T - 1)
            emit_drain(T - 1)
    nc.compile()
    return nc


LAST_RESULTS = None


def _make_consts():
    wp = np.zeros((P, 8 * P), dtype=np.float16)
    for k in range(8):
        wp[:, k * P:(k + 1) * P][np.arange(P), np.arange(P)] = \
            np.float16(5.0 * 2 ** k)
    ih = np.zeros((P, P), dtype=np.float16)
    ih[np.arange(P), np.arange(P)] = np.float16(1.0)
    return wp.view(np.uint16), ih.view(np.uint16)


def kernel(tx):
    global LAST_RESULTS
    tx = np.asarray(tx)
    assert tx.shape == (T, B, N) and tx.dtype == np.float32

    if "nc" not in _BUILT:
        _BUILT["nc"] = _build_nc()
    nc = _BUILT["nc"]

    wpack, ih = _make_consts()
    in_maps = []
    for c in range(N_CORES):
        xc = tx[:, c * B_SH:(c + 1) * B_SH, :].reshape(T, P, F)
        xc = np.ascontiguousarray(xc.transpose(1, 0, 2))     # [P, T, F]
        xh = xc.astype(np.float16).reshape(P, T * F)
        in_maps.append({
            "xs": xh.view(np.uint16),
            "wpack": wpack, "identh": ih,
        })

    res = run_bass_kernel_spmd(nc, in_maps, core_ids=list(range(N_CORES)),
                               trace=TRACE)
    LAST_RESULTS = res

    out = np.empty((T, B, N), dtype=np.float32)
    for c in range(N_CORES):
        yb = np.asarray(res.results[c]["y"]).astype(np.int16)  # [P, GROUPS*F]
        Bv = (yb + 128).astype(np.uint8).reshape(P, GROUPS, F)
        for g in range(GROUPS):
            for k in range(8):
                bits = (Bv[:, g, :] >> k) & 1          # [P, F]
                st = bits.reshape(B_SH, N).astype(np.float32)
                out[g * 8 + k, c * B_SH:(c + 1) * B_SH, :] = st
    return out


# revision 9
# speedup vs baseline: 1.4719x; 1.0072x over previous
"""LIF (leaky integrate-and-fire) forward kernel for Trainium2, 8 NeuronCores.

Recurrence (per element of [B, N], serial over T):
    v_t = DECAY * (v_{t-1} * (1 - s_{t-1})) + x_t      (REST = 0)
    s_t = (v_t > THRESHOLD)

v3.3 design: x is fed in fp16 (2 B/elem -> DMA floor ~47us/core).  The
carried state is u = DECAY * v * (v <= THR) in fp16, so the update is
v' = u + x.  The per-step spike indicator q = (v <= THR) * DECAY is 0 on
spike and ~0.2 otherwise; q doubles as the reset multiplier (u = q * v)
and as the packed-output source, so no Sign pass exists anywhere.

Two lane types over the per-core [128, 2048] slab:
  S-lane cols [0, C_S):    TensorE does the add in PSUM
      psv = I_f16^T x_f16 + I_f16^T u_f16      (2 matmuls, x one step early)
      v   = Identity(psv)     ScalarE -> fp16 SBUF
      q   = (v <= THR)*DECAY  DVE tensor_scalar (4x mode)
      u   = q * v             DVE tensor_tensor (2x mode)
  E-lane cols [C_S, F):    pure fp16 DVE
      v = u + x               DVE tensor_tensor (2x)
      q = (v <= THR)*DECAY    DVE tensor_scalar (4x)
      u = q * v               DVE tensor_tensor (2x)

Output: TensorE packs 8 steps of q into a PSUM byte-plane
(psum += 5*2^k I^T q_k ~= 2^k (1 - s_k)), one step deferred; ScalarE
drains it as int8 via out = -psum + 127 = S - 128 where S = sum 2^k s_k.
Host adds 128 and unpacks bits.
"""

import numpy as np

import concourse.bacc as bacc
import concourse.mybir as mybir
from concourse.tile import TileContext
from concourse.bass_utils import run_bass_kernel_spmd

T, B, N = 32, 128, 16384
N_CORES = 8
B_SH = B // N_CORES          # 16 batch rows per core
S = B_SH * N                 # 262144 elements per core per time step
P = 128                      # SBUF partitions
F = S // P                   # 2048 free-dim elements
DECAY = 0.2
THR = 0.3

C_S = 1536                   # PSUM-lane columns (multiple of 512)
C_E = F - C_S                # fp16 DVE-lane columns
N_SSUB = C_S // 512          # psv sub-lanes
CHUNK = 4                    # time steps per input DMA
GROUPS = T // 8              # byte-planes (8 steps packed per byte)

TRACE = False                # set True (e.g. from test.py) to capture a profile

_BUILT = {}


def _build_nc():
    nc = bacc.Bacc("TRN2", debug=False, num_devices=N_CORES)
    f16 = mybir.dt.float16
    f32 = mybir.dt.float32
    i8 = mybir.dt.int8
    Alu = mybir.AluOpType
    Act = mybir.ActivationFunctionType

    xs = nc.dram_tensor("xs", [P, T * F], f16, kind="ExternalInput").ap()
    wp_in = nc.dram_tensor("wpack", [P, 8 * P], f16, kind="ExternalInput").ap()
    ih_in = nc.dram_tensor("identh", [P, P], f16, kind="ExternalInput").ap()
    y = nc.dram_tensor("y", [P, GROUPS * F], i8, kind="ExternalOutput").ap()
    xr = xs.rearrange("p (t f) -> p t f", t=T)
    yr = y.rearrange("p (g f) -> p g f", g=GROUPS)

    E0 = C_S                 # E-lane column offset

    with TileContext(nc) as tc:
        with (
            tc.tile_pool(name="consts", bufs=1) as c_pool,
            tc.tile_pool(name="state", bufs=1) as st_pool,
            tc.tile_pool(name="xin", bufs=3) as xin_pool,
            tc.tile_pool(name="vs", bufs=3) as vs_pool,
            tc.tile_pool(name="qs", bufs=3) as qs_pool,
            tc.tile_pool(name="ve", bufs=3) as ve_pool,
            tc.tile_pool(name="qe", bufs=3) as qe_pool,
            tc.tile_pool(name="outs", bufs=2) as o_pool,
            tc.tile_pool(name="psv0", bufs=1, space="PSUM") as pv0_pool,
            tc.tile_pool(name="psv1", bufs=1, space="PSUM") as pv1_pool,
            tc.tile_pool(name="psv2", bufs=1, space="PSUM") as pv2_pool,
            tc.tile_pool(name="pack", bufs=1, space="PSUM") as pk_pool,
        ):
            zerob = nc.alloc_sbuf_tensor("const_zerob", [P, 1], f32).ap()
            nc.gpsimd.memset(zerob, 0.0)
            pos127 = nc.alloc_sbuf_tensor("const_pos127", [P, 1], f32).ap()
            nc.gpsimd.memset(pos127, 127.0)

            u_s = st_pool.tile([P, max(C_S, 1)], f16)    # S-lane state
            u_e = st_pool.tile([P, max(C_E, 1)], f16)    # E-lane state

            pv_pools = (pv0_pool, pv1_pool, pv2_pool)[:N_SSUB]

            def emit_pack(q_tiles, t_prev):
                # q_tiles: list of (ap, width) covering F columns of step
                # t_prev, in column order
                k = t_prev % 8
                wk = wsb[:, k * P:(k + 1) * P]
                col = 0
                for ap, wdt in q_tiles:
                    for r in range(0, wdt, 512):
                        nc.tensor.matmul(
                            out=pack_psum[:, col + r:col + r + 512], lhsT=wk,
                            rhs=ap[:, r:r + 512],
                            start=(k == 0), stop=(k == 7),
                        )
                    col += wdt

            def emit_drain(t_prev):
                g8 = t_prev // 8
                oi = o_pool.tile([P, F], i8, name="oi")
                nc.scalar.activation(
                    oi[:], pack_psum[:], Act.Identity,
                    bias=pos127, scale=-1.0)
                nc.scalar.dma_start(out=yr[:, g8, :], in_=oi[:])

            # --- prologue: first x slice, consts, rest of chunk 0 ---
            xt = xin_pool.tile([P, CHUNK * F], f16, name="xt")
            nc.sync.dma_start(out=xt[:, :F], in_=xr[:, 0, :])
            identh = c_pool.tile([P, P], f16)
            nc.sync.dma_start(out=identh[:], in_=ih_in)
            wsb = c_pool.tile([P, 8 * P], f16)
            nc.sync.dma_start(out=wsb[:], in_=wp_in)
            for jj in range(1, CHUNK):
                nc.sync.dma_start(
                    out=xt[:, jj * F:(jj + 1) * F], in_=xr[:, jj, :])

            q_prev = None
            pack_psum = None
            psvs = None

            def x_feed(tn, xtile):
                """x-feed matmuls for step tn into fresh psv tiles."""
                jn = tn % CHUNK
                tiles = [pool.tile([P, 512], f32, name=f"ps{i}")
                         for i, pool in enumerate(pv_pools)]
                for i in range(N_SSUB):
                    nc.tensor.matmul(
                        out=tiles[i][:], lhsT=identh[:],
                        rhs=xtile[:, jn * F + i * 512:jn * F + (i + 1) * 512],
                        start=True, stop=(tn == 0))
                return tiles

            psvs = x_feed(0, xt)

            for t in range(T):
                j = t % CHUNK
                # (chunk tiles for t>0 are loaded at the bottom of the
                # previous iteration so their x-feed can be emitted early)
                xe = xt[:, j * F + E0:j * F + E0 + C_E]  # E-lane x

                if t % 8 == 0:
                    pack_psum = pk_pool.tile([P, F], f32, name="pk")

                # --- PE: pack of t-1, u-feed of t, x-feed of t+1 ---
                if q_prev is not None:
                    emit_pack(q_prev, t - 1)
                if t > 0:
                    for i in range(N_SSUB):
                        nc.tensor.matmul(
                            out=psvs[i][:], lhsT=identh[:],
                            rhs=u_s[:, i * 512:(i + 1) * 512],
                            start=False, stop=True)
                cur_psvs = psvs
                if t + 1 < T:
                    nxt = xt if (t + 1) % CHUNK != 0 else None
                    # next chunk tile isn't allocated yet when j==CHUNK-1;
                    # defer that x-feed to the top of the next iteration
                    if nxt is not None:
                        psvs = x_feed(t + 1, nxt)
                    else:
                        psvs = None

                # --- E-lane first on DVE (independent of ScalarE copies) ---
                if t == 0:
                    ve = xe
                else:
                    vet = ve_pool.tile([P, C_E], f16, name="ve")
                    nc.vector.tensor_tensor(
                        out=vet[:], in0=u_e[:], in1=xe, op=Alu.add)
                    ve = vet[:]
                qe = qe_pool.tile([P, C_E], f16, name="qe")
                nc.vector.tensor_scalar(
                    out=qe[:], in0=ve, scalar1=THR, scalar2=DECAY,
                    op0=Alu.is_le, op1=Alu.mult)
                nc.vector.tensor_tensor(
                    out=u_e[:], in0=qe[:], in1=ve, op=Alu.mult)

                # --- S-lane: ScalarE copy + DVE fast ops ---
                qss = []
                for i in range(N_SSUB):
                    vs = vs_pool.tile([P, 512], f16, name=f"vs{i}")
                    nc.scalar.activation(
                        vs[:], cur_psvs[i][:], Act.Identity, bias=zerob)
                    qs = qs_pool.tile([P, 512], f16, name=f"qs{i}")
                    nc.vector.tensor_scalar(
                        out=qs[:], in0=vs[:], scalar1=THR, scalar2=DECAY,
                        op0=Alu.is_le, op1=Alu.mult)
                    nc.vector.tensor_tensor(
                        out=u_s[:, i * 512:(i + 1) * 512], in0=qs[:],
                        in1=vs[:], op=Alu.mult)
                    qss.append((qs, 512))

                # group drain (once per 8 steps), ordered last on ScalarE
                if t > 0 and (t - 1) % 8 == 7:
                    emit_drain(t - 1)

                q_prev = qss + [(qe, C_E)]

                # if the next step starts a new chunk, allocate+load it now
                # and emit its x-feed so PE stays ahead
                if t + 1 < T and (t + 1) % CHUNK == 0:
                    xt = xin_pool.tile([P, CHUNK * F], f16, name="xt")
                    nc.sync.dma_start(
                        out=xt[:], in_=xr[:, t + 1:t + 1 + CHUNK, :])
                    psvs = x_feed(t + 1, xt)

            emit_pack(q_prev, T - 1)
            emit_drain(T - 1)
    nc.compile()
    return nc


LAST_RESULTS = None


def _make_consts():
    wp = np.zeros((P, 8 * P), dtype=np.float16)
    for k in range(8):
        wp[:, k * P:(k + 1) * P][np.arange(P), np.arange(P)] = \
            np.float16(5.0 * 2 ** k)
    ih = np.zeros((P, P), dtype=np.float16)
    ih[np.arange(P), np.arange(P)] = np.float16(1.0)
    return wp.view(np.uint16), ih.view(np.uint16)


def kernel(tx):
    global LAST_RESULTS
    tx = np.asarray(tx)
    assert tx.shape == (T, B, N) and tx.dtype == np.float32

    if "nc" not in _BUILT:
        _BUILT["nc"] = _build_nc()
    nc = _BUILT["nc"]

    wpack, ih = _make_consts()
    in_maps = []
    for c in range(N_CORES):
        xc = tx[:, c * B_SH:(c + 1) * B_SH, :].reshape(T, P, F)
        xc = np.ascontiguousarray(xc.transpose(1, 0, 2))     # [P, T, F]
        xh = xc.astype(np.float16).reshape(P, T * F)
        in_maps.append({
            "xs": xh.view(np.uint16),
            "wpack": wpack, "identh": ih,
        })

    res = run_bass_kernel_spmd(nc, in_maps, core_ids=list(range(N_CORES)),
                               trace=TRACE)
    LAST_RESULTS = res

    out = np.empty((T, B, N), dtype=np.float32)
    for c in range(N_CORES):
        yb = np.asarray(res.results[c]["y"]).astype(np.int16)  # [P, GROUPS*F]
        Bv = (yb + 128).astype(np.uint8).reshape(P, GROUPS, F)
        for g in range(GROUPS):
            for k in range(8):
                bits = (Bv[:, g, :] >> k) & 1          # [P, F]
                st = bits.reshape(B_SH, N).astype(np.float32)
                out[g * 8 + k, c * B_SH:(c + 1) * B_SH, :] = st
    return out
